# revision 30
# baseline (speedup 1.0000x reference)
"""Trainium2 Bass kernel for nn_AttentionHAN (histogram_binning).

Strategy
--------
The reference network collapses algebraically:
  - t_K is dead; t_Q/i_Q/i_K and the output projection fold into small
    input-space matrices (computed on host from the replicated params).
  - Per batch row the device only needs 13 values:
      sp(4)  = pre-sigmoid attention scores
      tvd(4) = per-head dot of t_V with Wout[0,:128]
      ivd(4) = per-head dot of i_V with Wout[0,:128]
      base(1)= contribution of [t_Q, i_Q] @ Wout[0,128:] + bout
    plus the chi-square statistics of t_V/i_V, which reduce to per-feature
    counts S = #(v > thr) and C = #(v > thr and label==1).
  - out[b] = base + sum_h [ at*m1 + ai*m2 - (at*ai)*m3 ],
      m1 = s*tvd, m2 = s*ivd, m3 = s*m2,  s = sigmoid(sp),
    where at/ai = alpha_t/alpha_i depend on the GLOBAL chi statistics.

Sharding: pure data parallel over B on 8 cores (16384 rows each).  The tiny
per-core (128,) count tables are reduced on host (the "all-reduce" of the
sharding hint), alpha is computed exactly as the reference does, and a second
small kernel applies the 13-coefficient combination per row.

Launch A (per core, feature-on-partition, fp32r matmuls):
  tv.T(128f,512b)/iv.T accumulated over K=256 in PSUM; one DVE tensor_scalar
  (is_gt, per-partition threshold, accum_out) both binarizes and emits the
  per-block S count column; a K=1 PE matmul broadcasts the label row across
  partitions and tensor_tensor_reduce emits the per-block C column; the
  sm.T(13,512) matmul + ACT Identity(+bias) emits the 13-row R tensor.
Launch B (per core): PE-transposes R blocks to batch-on-partition, applies
  sigmoid + the coefficient combination.

All matmul operands use float32r (tf32 input rounding, exact products, fp32
accumulation); the host pre-rounds inputs so device numerics are
deterministic.  End-to-end error vs the fp32 reference is ~3e-4.
"""

import sys
import numpy as np

sys.path.insert(0, "/opt/trn_rl_repo")

import concourse.bacc as bacc  # noqa: E402
import concourse.tile as tile  # noqa: E402
from concourse import mybir  # noqa: E402

F32 = mybir.dt.float32
F32R = mybir.dt.float32r
f32 = np.float32


def _tf32(a):
    """Round-to-nearest-even to the tf32 grid (fp32r input quantization)."""
    u = np.ascontiguousarray(a, dtype=np.float32).view(np.uint32)
    add = np.uint32(0x00001000) + ((u >> np.uint32(13)) & np.uint32(1))
    return ((u + add) & np.uint32(0xFFFFE000)).view(np.float32)


B_TOT = 131072
IN = 256
HID = 128
H = 4
D = 32
NCORES = 8
THRESH = 0.7
BLK = 512
RPC = B_TOT // NCORES          # 16384 rows per core
NBLK = RPC // BLK              # 32 blocks of 512
SUPER = [2048] * 8             # kernel A DMA superblock sizes (sum = RPC)
SUPER_B = [4096] * 4           # kernel B superblock/group sizes (sum = RPC)
XBUFS = 3                      # kernel A x-tile buffering depth

_cache = {}


def _build_kernel_a():
    nc = bacc.Bacc("TRN2", target_bir_lowering=False, debug=False)
    xt = nc.dram_tensor("xt", (IN, RPC), F32R, kind="ExternalInput")
    xi = nc.dram_tensor("xi", (IN, RPC), F32R, kind="ExternalInput")
    lab = nc.dram_tensor("lab", (1, RPC), F32R, kind="ExternalInput")
    ones = nc.dram_tensor("ones", (1, 128), F32R, kind="ExternalInput")
    wtv = nc.dram_tensor("wtv", (IN, HID), F32R, kind="ExternalInput")
    wiv = nc.dram_tensor("wiv", (IN, HID), F32R, kind="ExternalInput")
    wsmt = nc.dram_tensor("wsmt", (IN, 13), F32R, kind="ExternalInput")
    wsmi = nc.dram_tensor("wsmi", (IN, 13), F32R, kind="ExternalInput")
    thrt = nc.dram_tensor("thrt", (HID, 1), F32, kind="ExternalInput")
    thri = nc.dram_tensor("thri", (HID, 1), F32, kind="ExternalInput")
    bsm = nc.dram_tensor("bsm", (13, 1), F32, kind="ExternalInput")
    r_out = nc.dram_tensor("r_out", (13, RPC), F32, kind="ExternalOutput")
    st_out = nc.dram_tensor("st_out", (HID, NBLK), F32, kind="ExternalOutput")
    si_out = nc.dram_tensor("si_out", (HID, NBLK), F32, kind="ExternalOutput")
    ct_out = nc.dram_tensor("ct_out", (HID, NBLK), F32, kind="ExternalOutput")
    ci_out = nc.dram_tensor("ci_out", (HID, NBLK), F32, kind="ExternalOutput")

    sb_max = max(SUPER)
    with tile.TileContext(nc) as tc:
        with (
            tc.tile_pool(name="w", bufs=1) as wp,
            tc.tile_pool(name="x", bufs=XBUFS) as xp,
            tc.tile_pool(name="fv", bufs=3) as fp,
            tc.tile_pool(name="acc", bufs=1) as ap,
            tc.tile_pool(name="rout", bufs=3) as rp,
            tc.tile_pool(name="ptv", bufs=2, space="PSUM") as ptvp,
            tc.tile_pool(name="piv", bufs=2, space="PSUM") as pivp,
            tc.tile_pool(name="psm", bufs=2, space="PSUM") as psmp,
            tc.tile_pool(name="plab", bufs=2, space="PSUM") as plabp,
        ):
            wtv_sb = [wp.tile([128, HID], F32R, name=f"wtv{k}", tag=f"wtv{k}")
                      for k in range(2)]
            wiv_sb = [wp.tile([128, HID], F32R, name=f"wiv{k}", tag=f"wiv{k}")
                      for k in range(2)]
            wsmt_sb = [wp.tile([128, 13], F32R, name=f"wsmt{k}", tag=f"wsmt{k}")
                       for k in range(2)]
            wsmi_sb = [wp.tile([128, 13], F32R, name=f"wsmi{k}", tag=f"wsmi{k}")
                       for k in range(2)]
            for k in range(2):
                sl = slice(k * 128, (k + 1) * 128)
                nc.sync.dma_start(wtv_sb[k][:], wtv[sl, :])
                nc.sync.dma_start(wiv_sb[k][:], wiv[sl, :])
                nc.sync.dma_start(wsmt_sb[k][:], wsmt[sl, :])
                nc.sync.dma_start(wsmi_sb[k][:], wsmi[sl, :])
            thrt_sb = wp.tile([HID, 1], F32, tag="thrt")
            thri_sb = wp.tile([HID, 1], F32, tag="thri")
            bsm_sb = wp.tile([13, 1], F32, tag="bsm")
            ones_sb = wp.tile([1, 128], F32R, tag="ones")
            nc.sync.dma_start(thrt_sb[:], thrt[:])
            nc.sync.dma_start(thri_sb[:], thri[:])
            nc.sync.dma_start(bsm_sb[:], bsm[:])
            nc.sync.dma_start(ones_sb[:], ones[:])

            st_sb = ap.tile([HID, NBLK], F32, tag="st")
            si_sb = ap.tile([HID, NBLK], F32, tag="si")
            ct_sb = ap.tile([HID, NBLK], F32, tag="ct")
            ci_sb = ap.tile([HID, NBLK], F32, tag="ci")

            blk = 0
            off = 0
            for size in SUPER:
                xt0 = xp.tile([128, sb_max], F32R, tag="xt0")
                xt1 = xp.tile([128, sb_max], F32R, tag="xt1")
                xi0 = xp.tile([128, sb_max], F32R, tag="xi0")
                xi1 = xp.tile([128, sb_max], F32R, tag="xi1")
                lab_sb = xp.tile([1, sb_max], F32R, tag="lab")
                nc.sync.dma_start(xt0[:, :size], xt[0:128, off:off + size])
                nc.sync.dma_start(xt1[:, :size], xt[128:256, off:off + size])
                nc.sync.dma_start(xi0[:, :size], xi[0:128, off:off + size])
                nc.sync.dma_start(xi1[:, :size], xi[128:256, off:off + size])
                nc.sync.dma_start(lab_sb[:, :size], lab[:, off:off + size])
                rt = rp.tile([13, sb_max], F32, tag="rt")
                for j in range(size // BLK):
                    o = j * BLK
                    ptv = ptvp.tile([128, BLK], F32)
                    piv = pivp.tile([128, BLK], F32)
                    psm = psmp.tile([13, BLK], F32)
                    plab = plabp.tile([128, BLK], F32)
                    nc.tensor.matmul(ptv[:], wtv_sb[0][:], xt0[:, o:o + BLK],
                                     start=True, stop=False)
                    nc.tensor.matmul(ptv[:], wtv_sb[1][:], xt1[:, o:o + BLK],
                                     start=False, stop=True)
                    nc.tensor.matmul(piv[:], wiv_sb[0][:], xi0[:, o:o + BLK],
                                     start=True, stop=False)
                    nc.tensor.matmul(piv[:], wiv_sb[1][:], xi1[:, o:o + BLK],
                                     start=False, stop=True)
                    nc.tensor.matmul(psm[:], wsmt_sb[0][:], xt0[:, o:o + BLK],
                                     start=True, stop=False)
                    nc.tensor.matmul(psm[:], wsmt_sb[1][:], xt1[:, o:o + BLK],
                                     start=False, stop=False)
                    nc.tensor.matmul(psm[:], wsmi_sb[0][:], xi0[:, o:o + BLK],
                                     start=False, stop=False)
                    nc.tensor.matmul(psm[:], wsmi_sb[1][:], xi1[:, o:o + BLK],
                                     start=False, stop=True)
                    # broadcast the label row across all 128 partitions
                    nc.tensor.matmul(plab[:], ones_sb[:], lab_sb[:, o:o + BLK],
                                     start=True, stop=True)
                    fvt = fp.tile([128, BLK], F32, tag="fvt")
                    fvi = fp.tile([128, BLK], F32, tag="fvi")
                    fvl = fp.tile([128, BLK], F32, tag="fvl")
                    # binarize + S count in one op
                    nc.vector.tensor_scalar(
                        fvt[:], ptv[:], thrt_sb[:], None,
                        op0=mybir.AluOpType.is_gt, op1=mybir.AluOpType.add,
                        accum_out=st_sb[:, blk:blk + 1])
                    nc.vector.tensor_scalar(
                        fvi[:], piv[:], thri_sb[:], None,
                        op0=mybir.AluOpType.is_gt, op1=mybir.AluOpType.add,
                        accum_out=si_sb[:, blk:blk + 1])
                    # C count: fv * label, reduced along batch (one PSUM
                    # operand max per DVE op -> fv from SBUF, label from PSUM)
                    nc.vector.scalar_tensor_tensor(
                        fvl[:], fvt[:], 1.0, plab[:],
                        op0=mybir.AluOpType.mult, op1=mybir.AluOpType.mult,
                        accum_out=ct_sb[:, blk:blk + 1])
                    nc.vector.scalar_tensor_tensor(
                        fvl[:], fvi[:], 1.0, plab[:],
                        op0=mybir.AluOpType.mult, op1=mybir.AluOpType.mult,
                        accum_out=ci_sb[:, blk:blk + 1])
                    nc.scalar.activation(
                        rt[:, o:o + BLK], psm[:],
                        mybir.ActivationFunctionType.Identity, bias=bsm_sb[:])
                    blk += 1
                nc.sync.dma_start(r_out[:, off:off + size], rt[:, :size])
                off += size

            nc.sync.dma_start(st_out[:], st_sb[:])
            nc.sync.dma_start(si_out[:], si_sb[:])
            nc.sync.dma_start(ct_out[:], ct_sb[:])
            nc.sync.dma_start(ci_out[:], ci_sb[:])

    nc.compile()
    return nc


def _build_kernel_b():
    nc = bacc.Bacc("TRN2", target_bir_lowering=False, debug=False)
    r = nc.dram_tensor("r", (13, RPC), F32, kind="ExternalInput")
    ident = nc.dram_tensor("ident", (128, 128), F32, kind="ExternalInput")
    crep = nc.dram_tensor("crep", (128, 384), F32, kind="ExternalInput")
    o_out = nc.dram_tensor("o_out", (128, NBLK * 4), F32, kind="ExternalOutput")

    sb_max = max(SUPER_B)
    with tile.TileContext(nc) as tc:
        with (
            tc.tile_pool(name="w", bufs=1) as wp,
            tc.tile_pool(name="r", bufs=2) as rp,
            tc.tile_pool(name="t", bufs=3) as tp,
            tc.tile_pool(name="out", bufs=1) as op,
            tc.tile_pool(name="ptr", bufs=2, space="PSUM") as pp,
        ):
            id_sb = wp.tile([128, 128], F32, tag="ident")
            nc.sync.dma_start(id_sb[:], ident[:])
            crep_sb = wp.tile([128, 384], F32, tag="crep")
            nc.sync.dma_start(crep_sb[:], crep[:])
            out_sb = op.tile([128, NBLK * 4], F32, tag="o")

            blk = 0
            off = 0
            for size in SUPER_B:
                rt = rp.tile([13, sb_max], F32, tag="rt")
                nc.sync.dma_start(rt[:, :size], r[:, off:off + size])
                g = size // BLK
                nch = 4 * g      # 128-row chunks in this group
                ptr = pp.tile([128, 52 * g], F32)
                for c in range(nch):
                    nc.tensor.transpose(
                        ptr[:, c * 13:(c + 1) * 13],
                        rt[0:13, c * 128:(c + 1) * 128],
                        id_sb[0:13, 0:13])
                p3 = ptr[:].rearrange("p (g k) -> p g k", k=13)
                s = tp.tile([128, 4 * nch], F32, tag="s")
                s3 = s[:].rearrange("p (g k) -> p g k", k=4)
                nc.scalar.activation(
                    s3, p3[:, :, 0:4], mybir.ActivationFunctionType.Sigmoid)
                m = tp.tile([128, 12 * nch], F32, tag="m")
                m3 = m[:].rearrange("p (g k) -> p g k", k=12)
                nc.vector.tensor_tensor(
                    m3[:, :, 0:4], s3, p3[:, :, 4:8], op=mybir.AluOpType.mult)
                nc.vector.tensor_tensor(
                    m3[:, :, 4:8], s3, p3[:, :, 8:12], op=mybir.AluOpType.mult)
                nc.vector.tensor_tensor(
                    m3[:, :, 8:12], s3, m3[:, :, 4:8], op=mybir.AluOpType.mult)
                mc = tp.tile([128, 12 * nch], F32, tag="mc")
                mc3 = mc[:].rearrange("p (g k) -> p g k", k=12)
                nc.vector.tensor_tensor(
                    mc[:], m[:], crep_sb[:, 0:12 * nch], op=mybir.AluOpType.mult)
                red = tp.tile([128, nch], F32, tag="red")
                nc.vector.tensor_reduce(
                    red[:], mc3, axis=mybir.AxisListType.X,
                    op=mybir.AluOpType.add)
                nc.vector.tensor_tensor(
                    out_sb[:, blk * 4:blk * 4 + nch], red[:], p3[:, :, 12],
                    op=mybir.AluOpType.add)
                blk += g
                off += size

            nc.sync.dma_start(o_out[:], out_sb[:])

    nc.compile()
    return nc


def _get_kernels():
    if "a" not in _cache:
        _cache["a"] = _build_kernel_a()
        _cache["b"] = _build_kernel_b()
    return _cache["a"], _cache["b"]


class _Runner:
    """Persistent jitted SPMD executor for a compiled Bass module.

    Mirrors bass2jax.run_bass_via_pjrt but keeps the jitted callable alive so
    repeated kernel() invocations skip retracing/recompilation."""

    def __init__(self, nc):
        import jax
        from jax.sharding import Mesh, PartitionSpec
        from jax.experimental.shard_map import shard_map
        from concourse import bass2jax

        bass2jax.install_neuronx_cc_hook()
        self._nc = nc
        pname = nc.partition_id_tensor.name if nc.partition_id_tensor else None
        in_names, out_names, out_avals = [], [], []
        self._zero_outs = []
        for alloc in nc.m.functions[0].allocations:
            if not isinstance(alloc, mybir.MemoryLocationSet):
                continue
            nm = alloc.memorylocations[0].name
            if alloc.kind == "ExternalInput":
                if nm != pname:
                    in_names.append(nm)
            elif alloc.kind == "ExternalOutput":
                out_names.append(nm)
                shape = tuple(alloc.tensor_shape)
                dt = mybir.dt.np(alloc.dtype)
                out_avals.append(jax.core.ShapedArray(shape, dt))
                self._zero_outs.append(np.zeros(shape, dt))
        self._in_names = in_names
        self._out_names = out_names
        all_in_names = in_names + out_names + ([pname] if pname else [])

        def _body(*args):
            operands = list(args)
            if pname:
                operands.append(bass2jax.partition_id_tensor())
            outs = bass2jax._bass_exec_p.bind(
                *operands, out_avals=tuple(out_avals),
                in_names=tuple(all_in_names), out_names=tuple(out_names),
                lowering_input_output_aliases=(), sim_require_finite=True,
                sim_require_nnan=True, nc=nc)
            return tuple(outs)

        devices = jax.devices()[:NCORES]
        assert len(devices) == NCORES, f"need {NCORES} devices"
        mesh = Mesh(np.asarray(devices), ("core",))
        nio = len(in_names) + len(out_names)
        self._fn = jax.jit(
            shard_map(_body, mesh=mesh,
                      in_specs=(PartitionSpec("core"),) * nio,
                      out_specs=(PartitionSpec("core"),) * len(out_names),
                      check_rep=False),
            keep_unused=True)

    def __call__(self, in_maps):
        assert len(in_maps) == NCORES
        concat = [
            np.concatenate([np.asarray(m[n]) for m in in_maps], axis=0)
            for n in self._in_names
        ]
        concat += [
            np.zeros((NCORES * z.shape[0], *z.shape[1:]), z.dtype)
            for z in self._zero_outs
        ]
        out_arrs = self._fn(*concat)
        results = []
        for c in range(NCORES):
            d = {}
            for i, nm in enumerate(self._out_names):
                full = np.asarray(out_arrs[i])
                per = full.shape[0] // NCORES
                d[nm] = full[c * per:(c + 1) * per]
            results.append(d)
        return results


def _get_runners():
    if "ra" not in _cache:
        nc_a, nc_b = _get_kernels()
        _cache["ra"] = _Runner(nc_a)
        _cache["rb"] = _Runner(nc_b)
    return _cache["ra"], _cache["rb"]


def _fold_params(p):
    """Fold all network params into the device weight matrices (host, f64)."""
    Wout = p["Wout"].astype(np.float64)
    bout = p["bout"].astype(np.float64)
    attn_W = p["attn_W"].astype(np.float64)
    attn_b = p["attn_b"].astype(np.float64)
    W1 = Wout[0, :HID]          # fused part
    W2 = Wout[0, HID:2 * HID]   # t_Q part
    W3 = Wout[0, 2 * HID:]      # i_Q part

    # A_t[32h+d, h] = attn_W[h, d];  A_i[32h+d, h] = attn_W[h, 32+d]
    A_t = np.zeros((HID, H))
    A_i = np.zeros((HID, H))
    Bt = np.zeros((HID, H))
    for h in range(H):
        A_t[h * D:(h + 1) * D, h] = attn_W[h, :D]
        A_i[h * D:(h + 1) * D, h] = attn_W[h, D:]
        Bt[h * D:(h + 1) * D, h] = W1[h * D:(h + 1) * D]

    def WT(name):
        return p[name].astype(np.float64).T  # (IN, HID)

    wsmt = np.zeros((IN, 13))
    wsmt[:, 0:4] = WT("Wtq") @ A_t
    wsmt[:, 4:8] = WT("Wtv") @ Bt
    wsmt[:, 12] = WT("Wtq") @ W2
    wsmi = np.zeros((IN, 13))
    wsmi[:, 0:4] = WT("Wik") @ A_i
    wsmi[:, 8:12] = WT("Wiv") @ Bt
    wsmi[:, 12] = WT("Wiq") @ W3

    bsm = np.zeros(13)
    bsm[0:4] = (p["btq"].astype(np.float64) @ A_t
                + p["bik"].astype(np.float64) @ A_i + attn_b)
    bsm[4:8] = p["btv"].astype(np.float64) @ Bt
    bsm[8:12] = p["biv"].astype(np.float64) @ Bt
    bsm[12] = (p["btq"].astype(np.float64) @ W2
               + p["biq"].astype(np.float64) @ W3 + bout[0])

    thrt = f32(THRESH) - p["btv"].astype(f32)   # f32: matches device compare
    thri = f32(THRESH) - p["biv"].astype(f32)

    return {
        "wtv": _tf32(np.ascontiguousarray(WT("Wtv"), dtype=f32)),
        "wiv": _tf32(np.ascontiguousarray(WT("Wiv"), dtype=f32)),
        "wsmt": _tf32(wsmt.astype(f32)),
        "wsmi": _tf32(wsmi.astype(f32)),
        "thrt": thrt.reshape(HID, 1),
        "thri": thri.reshape(HID, 1),
        "bsm": bsm.astype(f32).reshape(13, 1),
        "ones": np.ones((1, 128), dtype=f32),
    }


def _chi_square_from_counts(S, C, L, B):
    """Replicate the reference chi-square given exact integer counts (f32 ops)."""
    F = S.shape[0]
    counts = np.zeros((F, 2, 2), dtype=f32)
    counts[:, 1, 1] = C
    counts[:, 1, 0] = S - C
    counts[:, 0, 1] = L - C
    counts[:, 0, 0] = B - S - L + C
    total = counts.sum(axis=(1, 2), dtype=f32)
    col = counts.sum(axis=1, dtype=f32)   # (F,2) over f_val -> label counts
    row = counts.sum(axis=2, dtype=f32)   # (F,2) over l_val -> feature counts
    expected = col[:, :, None] * row[:, None, :] / (total[:, None, None] + f32(1e-6))
    chi = ((counts - expected) ** 2 / (expected + f32(1e-6))).sum(
        axis=(1, 2), dtype=f32)
    return chi


def kernel(**inputs):
    text = _tf32(np.asarray(inputs["text_vec"], dtype=f32))
    image = _tf32(np.asarray(inputs["image_vec"], dtype=f32))
    label = np.asarray(inputs["label"]).astype(np.int64)

    folded = _fold_params(inputs)
    run_a, run_b = _get_runners()

    lab_f = label.astype(f32).reshape(NCORES, 1, RPC)
    in_maps = []
    for c in range(NCORES):
        sl = slice(c * RPC, (c + 1) * RPC)
        m = {
            "xt": np.ascontiguousarray(text[sl].T),
            "xi": np.ascontiguousarray(image[sl].T),
            "lab": lab_f[c],
        }
        m.update(folded)
        in_maps.append(m)

    # ---- launch A
    res_a = run_a(in_maps)

    # ---- host: reduce the tiny count tables, compute alpha (the "all-reduce")
    S_t = np.zeros(HID)
    S_i = np.zeros(HID)
    C_t = np.zeros(HID)
    C_i = np.zeros(HID)
    for c in range(NCORES):
        S_t += res_a[c]["st_out"].astype(np.float64).sum(axis=1)
        S_i += res_a[c]["si_out"].astype(np.float64).sum(axis=1)
        C_t += res_a[c]["ct_out"].astype(np.float64).sum(axis=1)
        C_i += res_a[c]["ci_out"].astype(np.float64).sum(axis=1)
    L = float(label.sum())
    chi_t = _chi_square_from_counts(S_t, C_t, L, float(B_TOT))
    chi_i = _chi_square_from_counts(S_i, C_i, L, float(B_TOT))
    chi_max = f32(max(chi_t.max(), chi_i.max()))
    alpha_t = (chi_t / (chi_max + f32(1e-6)))[:H].astype(f32)
    alpha_i = (chi_i / (chi_max + f32(1e-6)))[:H].astype(f32)

    coeffs = np.concatenate([alpha_t, alpha_i, -(alpha_t * alpha_i)]).astype(f32)
    crep = np.tile(np.tile(coeffs, 32)[None, :], (128, 1)).astype(f32)
    ident = np.eye(128, dtype=f32)

    in_maps_b = [
        {"r": res_a[c]["r_out"], "ident": ident, "crep": crep}
        for c in range(NCORES)
    ]

    # ---- launch B
    res_b = run_b(in_maps_b)

    # ---- gather
    out = np.empty((B_TOT, 1), dtype=f32)
    for c in range(NCORES):
        o = res_b[c]["o_out"]  # (128, NBLK*4)
        rows = o.reshape(128, NBLK * 4).T.reshape(RPC)
        out[c * RPC:(c + 1) * RPC, 0] = rows
    return out


# revision 39
# speedup vs baseline: 1.1258x; 1.1258x over previous
"""Trainium2 Bass kernel for nn_AttentionHAN (histogram_binning).

Strategy
--------
The reference network collapses algebraically:
  - t_K is dead; t_Q/i_Q/i_K and the output projection fold into small
    input-space matrices (computed on host from the replicated params).
  - Per batch row the device only needs 13 values:
      sp(4)  = pre-sigmoid attention scores
      tvd(4) = per-head dot of t_V with Wout[0,:128]
      ivd(4) = per-head dot of i_V with Wout[0,:128]
      base(1)= contribution of [t_Q, i_Q] @ Wout[0,128:] + bout
    plus the chi-square statistics of t_V/i_V, which reduce to per-feature
    counts S = #(v > thr) and C = #(v > thr and label==1).
  - out[b] = base + sum_h [ at*m1 + ai*m2 - (at*ai)*m3 ],
      m1 = s*tvd, m2 = s*ivd, m3 = s*m2,  s = sigmoid(sp),
    where at/ai = alpha_t/alpha_i depend on the GLOBAL chi statistics.

Sharding: pure data parallel over B on 8 cores (16384 rows each).  The tiny
per-core (128,) count tables are reduced on host (the "all-reduce" of the
sharding hint), alpha is computed exactly as the reference does, and a second
small kernel applies the 13-coefficient combination per row.

Launch A (per core, feature-on-partition, fp32r matmuls):
  tv.T(128f,512b)/iv.T accumulated over K=256 in PSUM; one DVE tensor_scalar
  (is_gt, per-partition threshold, accum_out) both binarizes and emits the
  per-block S count column; a K=1 PE matmul broadcasts the label row across
  partitions and tensor_tensor_reduce emits the per-block C column; the
  sm.T(13,512) matmul + ACT Identity(+bias) emits the 13-row R tensor.
Launch B (per core): PE-transposes R blocks to batch-on-partition, applies
  sigmoid + the coefficient combination.

All matmul operands use float32r (tf32 input rounding, exact products, fp32
accumulation); the host pre-rounds inputs so device numerics are
deterministic.  End-to-end error vs the fp32 reference is ~3e-4.
"""

import sys
import numpy as np

sys.path.insert(0, "/opt/trn_rl_repo")

import concourse.bacc as bacc  # noqa: E402
import concourse.tile as tile  # noqa: E402
from concourse import mybir  # noqa: E402

F32 = mybir.dt.float32
F32R = mybir.dt.float32r
f32 = np.float32


def _tf32(a):
    """Round-to-nearest-even to the tf32 grid (fp32r input quantization)."""
    u = np.ascontiguousarray(a, dtype=np.float32).view(np.uint32)
    add = np.uint32(0x00001000) + ((u >> np.uint32(13)) & np.uint32(1))
    return ((u + add) & np.uint32(0xFFFFE000)).view(np.float32)


B_TOT = 131072
IN = 256
HID = 128
H = 4
D = 32
NCORES = 8
THRESH = 0.7
BLK = 512
RPC = B_TOT // NCORES          # 16384 rows per core
NBLK = RPC // BLK              # 32 blocks of 512
SUPER = [2048] * 8             # kernel A DMA superblock sizes (sum = RPC)
SUPER_B = [4096] * 4           # kernel B superblock/group sizes (sum = RPC)
XBUFS = 3                      # kernel A x-tile buffering depth

_cache = {}


def _build_kernel_a():
    nc = bacc.Bacc("TRN2", target_bir_lowering=False, debug=False)
    xt = nc.dram_tensor("xt", (IN, RPC), F32R, kind="ExternalInput")
    xi = nc.dram_tensor("xi", (IN, RPC), F32R, kind="ExternalInput")
    lab = nc.dram_tensor("lab", (1, BLK), F32R, kind="ExternalInput")
    ones = nc.dram_tensor("ones", (1, 128), F32R, kind="ExternalInput")
    wtv = nc.dram_tensor("wtv", (IN, HID), F32R, kind="ExternalInput")
    wiv = nc.dram_tensor("wiv", (IN, HID), F32R, kind="ExternalInput")
    wsmt = nc.dram_tensor("wsmt", (IN, 13), F32R, kind="ExternalInput")
    wsmi = nc.dram_tensor("wsmi", (IN, 13), F32R, kind="ExternalInput")
    thrt = nc.dram_tensor("thrt", (HID, 1), F32, kind="ExternalInput")
    thri = nc.dram_tensor("thri", (HID, 1), F32, kind="ExternalInput")
    bsm = nc.dram_tensor("bsm", (13, 1), F32, kind="ExternalInput")
    r_out = nc.dram_tensor("r_out", (13, RPC), F32, kind="ExternalOutput")
    st_out = nc.dram_tensor("st_out", (HID, NBLK), F32, kind="ExternalOutput")
    si_out = nc.dram_tensor("si_out", (HID, NBLK), F32, kind="ExternalOutput")
    ct_out = nc.dram_tensor("ct_out", (HID, 1), F32, kind="ExternalOutput")
    ci_out = nc.dram_tensor("ci_out", (HID, 1), F32, kind="ExternalOutput")

    sb_max = max(SUPER)
    with tile.TileContext(nc) as tc:
        with (
            tc.tile_pool(name="w", bufs=1) as wp,
            tc.tile_pool(name="x", bufs=XBUFS) as xp,
            tc.tile_pool(name="fv", bufs=3) as fp,
            tc.tile_pool(name="acc", bufs=1) as ap,
            tc.tile_pool(name="rout", bufs=3) as rp,
            tc.tile_pool(name="ptv", bufs=2, space="PSUM") as ptvp,
            tc.tile_pool(name="piv", bufs=2, space="PSUM") as pivp,
            tc.tile_pool(name="psm", bufs=2, space="PSUM") as psmp,
            tc.tile_pool(name="plab", bufs=1, space="PSUM") as plabp,
        ):
            wtv_sb = [wp.tile([128, HID], F32R, name=f"wtv{k}", tag=f"wtv{k}")
                      for k in range(2)]
            wiv_sb = [wp.tile([128, HID], F32R, name=f"wiv{k}", tag=f"wiv{k}")
                      for k in range(2)]
            wsmt_sb = [wp.tile([128, 13], F32R, name=f"wsmt{k}", tag=f"wsmt{k}")
                       for k in range(2)]
            wsmi_sb = [wp.tile([128, 13], F32R, name=f"wsmi{k}", tag=f"wsmi{k}")
                       for k in range(2)]
            for k in range(2):
                sl = slice(k * 128, (k + 1) * 128)
                nc.sync.dma_start(wtv_sb[k][:], wtv[sl, :])
                nc.sync.dma_start(wiv_sb[k][:], wiv[sl, :])
                nc.sync.dma_start(wsmt_sb[k][:], wsmt[sl, :])
                nc.sync.dma_start(wsmi_sb[k][:], wsmi[sl, :])
            thrt_sb = wp.tile([HID, 1], F32, tag="thrt")
            thri_sb = wp.tile([HID, 1], F32, tag="thri")
            bsm_sb = wp.tile([13, 1], F32, tag="bsm")
            ones_sb = wp.tile([1, 128], F32R, tag="ones")
            nc.sync.dma_start(thrt_sb[:], thrt[:])
            nc.sync.dma_start(thri_sb[:], thri[:])
            nc.sync.dma_start(bsm_sb[:], bsm[:])
            nc.sync.dma_start(ones_sb[:], ones[:])

            st_sb = ap.tile([HID, NBLK], F32, tag="st")
            si_sb = ap.tile([HID, NBLK], F32, tag="si")
            ct_sb = ap.tile([HID, 1], F32, tag="ct")
            ci_sb = ap.tile([HID, 1], F32, tag="ci")
            lab_sb = ap.tile([1, BLK], F32R, tag="lab")
            nc.sync.dma_start(lab_sb[:], lab[:])

            blk = 0
            off = 0
            for size in SUPER:
                xt0 = xp.tile([128, sb_max], F32R, tag="xt0")
                xt1 = xp.tile([128, sb_max], F32R, tag="xt1")
                xi0 = xp.tile([128, sb_max], F32R, tag="xi0")
                xi1 = xp.tile([128, sb_max], F32R, tag="xi1")
                nc.sync.dma_start(xt0[:, :size], xt[0:128, off:off + size])
                nc.sync.dma_start(xt1[:, :size], xt[128:256, off:off + size])
                nc.sync.dma_start(xi0[:, :size], xi[0:128, off:off + size])
                nc.sync.dma_start(xi1[:, :size], xi[128:256, off:off + size])
                rt = rp.tile([13, sb_max], F32, tag="rt")
                for j in range(size // BLK):
                    o = j * BLK
                    ptv = ptvp.tile([128, BLK], F32)
                    piv = pivp.tile([128, BLK], F32)
                    psm = psmp.tile([13, BLK], F32)
                    nc.tensor.matmul(ptv[:], wtv_sb[0][:], xt0[:, o:o + BLK],
                                     start=True, stop=False)
                    nc.tensor.matmul(ptv[:], wtv_sb[1][:], xt1[:, o:o + BLK],
                                     start=False, stop=True)
                    nc.tensor.matmul(piv[:], wiv_sb[0][:], xi0[:, o:o + BLK],
                                     start=True, stop=False)
                    nc.tensor.matmul(piv[:], wiv_sb[1][:], xi1[:, o:o + BLK],
                                     start=False, stop=True)
                    nc.tensor.matmul(psm[:], wsmt_sb[0][:], xt0[:, o:o + BLK],
                                     start=True, stop=False)
                    nc.tensor.matmul(psm[:], wsmt_sb[1][:], xt1[:, o:o + BLK],
                                     start=False, stop=False)
                    nc.tensor.matmul(psm[:], wsmi_sb[0][:], xi0[:, o:o + BLK],
                                     start=False, stop=False)
                    nc.tensor.matmul(psm[:], wsmi_sb[1][:], xi1[:, o:o + BLK],
                                     start=False, stop=True)
                    fvt = fp.tile([128, BLK], F32, tag="fvt")
                    fvi = fp.tile([128, BLK], F32, tag="fvi")
                    # binarize + S count in one op
                    nc.vector.tensor_scalar(
                        fvt[:], ptv[:], thrt_sb[:], None,
                        op0=mybir.AluOpType.is_gt, op1=mybir.AluOpType.add,
                        accum_out=st_sb[:, blk:blk + 1])
                    nc.vector.tensor_scalar(
                        fvi[:], piv[:], thri_sb[:], None,
                        op0=mybir.AluOpType.is_gt, op1=mybir.AluOpType.add,
                        accum_out=si_sb[:, blk:blk + 1])
                    if blk == NBLK - 1:
                        # the single possibly-mixed block: per-feature count
                        # of (v > thr) rows with label==1.  Label row is
                        # broadcast across partitions via a K=1 matmul; one
                        # PSUM operand max per DVE op -> fv from SBUF.
                        plab = plabp.tile([128, BLK], F32)
                        nc.tensor.matmul(plab[:], ones_sb[:], lab_sb[:],
                                         start=True, stop=True)
                        fvl = fp.tile([128, BLK], F32, tag="fvl")
                        nc.vector.scalar_tensor_tensor(
                            fvl[:], fvt[:], 1.0, plab[:],
                            op0=mybir.AluOpType.mult, op1=mybir.AluOpType.mult,
                            accum_out=ct_sb[:, 0:1])
                        nc.vector.scalar_tensor_tensor(
                            fvl[:], fvi[:], 1.0, plab[:],
                            op0=mybir.AluOpType.mult, op1=mybir.AluOpType.mult,
                            accum_out=ci_sb[:, 0:1])
                    nc.scalar.activation(
                        rt[:, o:o + BLK], psm[:],
                        mybir.ActivationFunctionType.Identity, bias=bsm_sb[:])
                    blk += 1
                nc.sync.dma_start(r_out[:, off:off + size], rt[:, :size])
                off += size

            nc.sync.dma_start(st_out[:], st_sb[:])
            nc.sync.dma_start(si_out[:], si_sb[:])
            nc.sync.dma_start(ct_out[:], ct_sb[:])
            nc.sync.dma_start(ci_out[:], ci_sb[:])

    nc.compile()
    return nc


def _build_kernel_b():
    nc = bacc.Bacc("TRN2", target_bir_lowering=False, debug=False)
    r = nc.dram_tensor("r", (13, RPC), F32, kind="ExternalInput")
    ident = nc.dram_tensor("ident", (128, 128), F32, kind="ExternalInput")
    crep = nc.dram_tensor("crep", (128, 384), F32, kind="ExternalInput")
    o_out = nc.dram_tensor("o_out", (128, NBLK * 4), F32, kind="ExternalOutput")

    sb_max = max(SUPER_B)
    with tile.TileContext(nc) as tc:
        with (
            tc.tile_pool(name="w", bufs=1) as wp,
            tc.tile_pool(name="r", bufs=2) as rp,
            tc.tile_pool(name="t", bufs=3) as tp,
            tc.tile_pool(name="out", bufs=1) as op,
            tc.tile_pool(name="ptr", bufs=2, space="PSUM") as pp,
        ):
            id_sb = wp.tile([128, 128], F32, tag="ident")
            nc.sync.dma_start(id_sb[:], ident[:])
            crep_sb = wp.tile([128, 384], F32, tag="crep")
            nc.sync.dma_start(crep_sb[:], crep[:])
            out_sb = op.tile([128, NBLK * 4], F32, tag="o")

            blk = 0
            off = 0
            for size in SUPER_B:
                rt = rp.tile([13, sb_max], F32, tag="rt")
                nc.sync.dma_start(rt[:, :size], r[:, off:off + size])
                g = size // BLK
                nch = 4 * g      # 128-row chunks in this group
                ptr = pp.tile([128, 52 * g], F32)
                for c in range(nch):
                    nc.tensor.transpose(
                        ptr[:, c * 13:(c + 1) * 13],
                        rt[0:13, c * 128:(c + 1) * 128],
                        id_sb[0:13, 0:13])
                p3 = ptr[:].rearrange("p (g k) -> p g k", k=13)
                s = tp.tile([128, 4 * nch], F32, tag="s")
                s3 = s[:].rearrange("p (g k) -> p g k", k=4)
                nc.scalar.activation(
                    s3, p3[:, :, 0:4], mybir.ActivationFunctionType.Sigmoid)
                m = tp.tile([128, 12 * nch], F32, tag="m")
                m3 = m[:].rearrange("p (g k) -> p g k", k=12)
                nc.vector.tensor_tensor(
                    m3[:, :, 0:4], s3, p3[:, :, 4:8], op=mybir.AluOpType.mult)
                nc.vector.tensor_tensor(
                    m3[:, :, 4:8], s3, p3[:, :, 8:12], op=mybir.AluOpType.mult)
                nc.vector.tensor_tensor(
                    m3[:, :, 8:12], s3, m3[:, :, 4:8], op=mybir.AluOpType.mult)
                mc = tp.tile([128, 12 * nch], F32, tag="mc")
                mc3 = mc[:].rearrange("p (g k) -> p g k", k=12)
                nc.vector.tensor_tensor(
                    mc[:], m[:], crep_sb[:, 0:12 * nch], op=mybir.AluOpType.mult)
                red = tp.tile([128, nch], F32, tag="red")
                nc.vector.tensor_reduce(
                    red[:], mc3, axis=mybir.AxisListType.X,
                    op=mybir.AluOpType.add)
                nc.vector.tensor_tensor(
                    out_sb[:, blk * 4:blk * 4 + nch], red[:], p3[:, :, 12],
                    op=mybir.AluOpType.add)
                blk += g
                off += size

            nc.sync.dma_start(o_out[:], out_sb[:])

    nc.compile()
    return nc


def _get_kernels():
    if "a" not in _cache:
        _cache["a"] = _build_kernel_a()
        _cache["b"] = _build_kernel_b()
    return _cache["a"], _cache["b"]


class _Runner:
    """Persistent jitted SPMD executor for a compiled Bass module.

    Mirrors bass2jax.run_bass_via_pjrt but keeps the jitted callable alive so
    repeated kernel() invocations skip retracing/recompilation."""

    def __init__(self, nc):
        import jax
        from jax.sharding import Mesh, PartitionSpec
        from jax.experimental.shard_map import shard_map
        from concourse import bass2jax

        bass2jax.install_neuronx_cc_hook()
        self._nc = nc
        pname = nc.partition_id_tensor.name if nc.partition_id_tensor else None
        in_names, out_names, out_avals = [], [], []
        self._zero_outs = []
        for alloc in nc.m.functions[0].allocations:
            if not isinstance(alloc, mybir.MemoryLocationSet):
                continue
            nm = alloc.memorylocations[0].name
            if alloc.kind == "ExternalInput":
                if nm != pname:
                    in_names.append(nm)
            elif alloc.kind == "ExternalOutput":
                out_names.append(nm)
                shape = tuple(alloc.tensor_shape)
                dt = mybir.dt.np(alloc.dtype)
                out_avals.append(jax.core.ShapedArray(shape, dt))
                self._zero_outs.append(np.zeros(shape, dt))
        self._in_names = in_names
        self._out_names = out_names
        all_in_names = in_names + out_names + ([pname] if pname else [])

        def _body(*args):
            operands = list(args)
            if pname:
                operands.append(bass2jax.partition_id_tensor())
            outs = bass2jax._bass_exec_p.bind(
                *operands, out_avals=tuple(out_avals),
                in_names=tuple(all_in_names), out_names=tuple(out_names),
                lowering_input_output_aliases=(), sim_require_finite=True,
                sim_require_nnan=True, nc=nc)
            return tuple(outs)

        devices = jax.devices()[:NCORES]
        assert len(devices) == NCORES, f"need {NCORES} devices"
        mesh = Mesh(np.asarray(devices), ("core",))
        nio = len(in_names) + len(out_names)
        self._fn = jax.jit(
            shard_map(_body, mesh=mesh,
                      in_specs=(PartitionSpec("core"),) * nio,
                      out_specs=(PartitionSpec("core"),) * len(out_names),
                      check_rep=False),
            keep_unused=True)

    def __call__(self, in_maps):
        assert len(in_maps) == NCORES
        concat = [
            np.concatenate([np.asarray(m[n]) for m in in_maps], axis=0)
            for n in self._in_names
        ]
        concat += [
            np.zeros((NCORES * z.shape[0], *z.shape[1:]), z.dtype)
            for z in self._zero_outs
        ]
        out_arrs = self._fn(*concat)
        results = []
        for c in range(NCORES):
            d = {}
            for i, nm in enumerate(self._out_names):
                full = np.asarray(out_arrs[i])
                per = full.shape[0] // NCORES
                d[nm] = full[c * per:(c + 1) * per]
            results.append(d)
        return results


def _get_runners():
    if "ra" not in _cache:
        nc_a, nc_b = _get_kernels()
        _cache["ra"] = _Runner(nc_a)
        _cache["rb"] = _Runner(nc_b)
    return _cache["ra"], _cache["rb"]


def _fold_params(p):
    """Fold all network params into the device weight matrices (host, f64)."""
    Wout = p["Wout"].astype(np.float64)
    bout = p["bout"].astype(np.float64)
    attn_W = p["attn_W"].astype(np.float64)
    attn_b = p["attn_b"].astype(np.float64)
    W1 = Wout[0, :HID]          # fused part
    W2 = Wout[0, HID:2 * HID]   # t_Q part
    W3 = Wout[0, 2 * HID:]      # i_Q part

    # A_t[32h+d, h] = attn_W[h, d];  A_i[32h+d, h] = attn_W[h, 32+d]
    A_t = np.zeros((HID, H))
    A_i = np.zeros((HID, H))
    Bt = np.zeros((HID, H))
    for h in range(H):
        A_t[h * D:(h + 1) * D, h] = attn_W[h, :D]
        A_i[h * D:(h + 1) * D, h] = attn_W[h, D:]
        Bt[h * D:(h + 1) * D, h] = W1[h * D:(h + 1) * D]

    def WT(name):
        return p[name].astype(np.float64).T  # (IN, HID)

    wsmt = np.zeros((IN, 13))
    wsmt[:, 0:4] = WT("Wtq") @ A_t
    wsmt[:, 4:8] = WT("Wtv") @ Bt
    wsmt[:, 12] = WT("Wtq") @ W2
    wsmi = np.zeros((IN, 13))
    wsmi[:, 0:4] = WT("Wik") @ A_i
    wsmi[:, 8:12] = WT("Wiv") @ Bt
    wsmi[:, 12] = WT("Wiq") @ W3

    bsm = np.zeros(13)
    bsm[0:4] = (p["btq"].astype(np.float64) @ A_t
                + p["bik"].astype(np.float64) @ A_i + attn_b)
    bsm[4:8] = p["btv"].astype(np.float64) @ Bt
    bsm[8:12] = p["biv"].astype(np.float64) @ Bt
    bsm[12] = (p["btq"].astype(np.float64) @ W2
               + p["biq"].astype(np.float64) @ W3 + bout[0])

    thrt = f32(THRESH) - p["btv"].astype(f32)   # f32: matches device compare
    thri = f32(THRESH) - p["biv"].astype(f32)

    return {
        "wtv": _tf32(np.ascontiguousarray(WT("Wtv"), dtype=f32)),
        "wiv": _tf32(np.ascontiguousarray(WT("Wiv"), dtype=f32)),
        "wsmt": _tf32(wsmt.astype(f32)),
        "wsmi": _tf32(wsmi.astype(f32)),
        "thrt": thrt.reshape(HID, 1),
        "thri": thri.reshape(HID, 1),
        "bsm": bsm.astype(f32).reshape(13, 1),
        "ones": np.ones((1, 128), dtype=f32),
    }


def _chi_square_from_counts(S, C, L, B):
    """Replicate the reference chi-square given exact integer counts (f32 ops)."""
    F = S.shape[0]
    counts = np.zeros((F, 2, 2), dtype=f32)
    counts[:, 1, 1] = C
    counts[:, 1, 0] = S - C
    counts[:, 0, 1] = L - C
    counts[:, 0, 0] = B - S - L + C
    total = counts.sum(axis=(1, 2), dtype=f32)
    col = counts.sum(axis=1, dtype=f32)   # (F,2) over f_val -> label counts
    row = counts.sum(axis=2, dtype=f32)   # (F,2) over l_val -> feature counts
    expected = col[:, :, None] * row[:, None, :] / (total[:, None, None] + f32(1e-6))
    chi = ((counts - expected) ** 2 / (expected + f32(1e-6))).sum(
        axis=(1, 2), dtype=f32)
    return chi


def kernel(**inputs):
    text = _tf32(np.asarray(inputs["text_vec"], dtype=f32))
    image = _tf32(np.asarray(inputs["image_vec"], dtype=f32))
    label = np.asarray(inputs["label"]).astype(np.int64)

    folded = _fold_params(inputs)
    run_a, run_b = _get_runners()

    # Row assignment: sort all rows by label, deal contiguous RPC-row chunks
    # to cores, then within each core rotate the (at most one) mixed 512-row
    # block to device block index 31, so blocks 0..30 are label-pure and only
    # block 31 needs the on-device label-weighted count.
    order = np.concatenate([np.flatnonzero(label == 0),
                            np.flatnonzero(label != 0)])
    in_maps = []
    srcs = []
    pure1_masks = []
    for c in range(NCORES):
        chunk = order[c * RPC:(c + 1) * RPC]
        n0 = int((label[chunk] == 0).sum())
        k0, r0 = divmod(n0, BLK)
        if r0 > 0:
            src = np.concatenate([chunk[0:k0 * BLK], chunk[(k0 + 1) * BLK:],
                                  chunk[k0 * BLK:(k0 + 1) * BLK]])
        else:
            src = chunk
        lab_perm = (label[src] != 0)
        blocks = lab_perm.reshape(NBLK, BLK)
        pure1 = blocks.all(axis=1)
        mixed = blocks.any(axis=1) & ~pure1
        assert not mixed[:NBLK - 1].any(), "mixed block must be at index 31"
        m = {
            "xt": np.ascontiguousarray(text[src].T),
            "xi": np.ascontiguousarray(image[src].T),
            "lab": lab_perm[NBLK * BLK - BLK:].astype(f32).reshape(1, BLK),
        }
        m.update(folded)
        in_maps.append(m)
        srcs.append(src)
        pure1_masks.append(pure1[:NBLK - 1])

    # ---- launch A
    res_a = run_a(in_maps)

    # ---- host: reduce the tiny count tables, compute alpha (the "all-reduce")
    S_t = np.zeros(HID)
    S_i = np.zeros(HID)
    C_t = np.zeros(HID)
    C_i = np.zeros(HID)
    for c in range(NCORES):
        st = res_a[c]["st_out"].astype(np.float64)   # (128, NBLK)
        si = res_a[c]["si_out"].astype(np.float64)
        S_t += st.sum(axis=1)
        S_i += si.sum(axis=1)
        p1 = pure1_masks[c]
        C_t += st[:, :NBLK - 1][:, p1].sum(axis=1) + res_a[c]["ct_out"][:, 0]
        C_i += si[:, :NBLK - 1][:, p1].sum(axis=1) + res_a[c]["ci_out"][:, 0]
    L = float((label != 0).sum())
    chi_t = _chi_square_from_counts(S_t, C_t, L, float(B_TOT))
    chi_i = _chi_square_from_counts(S_i, C_i, L, float(B_TOT))
    chi_max = f32(max(chi_t.max(), chi_i.max()))
    alpha_t = (chi_t / (chi_max + f32(1e-6)))[:H].astype(f32)
    alpha_i = (chi_i / (chi_max + f32(1e-6)))[:H].astype(f32)

    coeffs = np.concatenate([alpha_t, alpha_i, -(alpha_t * alpha_i)]).astype(f32)
    crep = np.tile(np.tile(coeffs, 32)[None, :], (128, 1)).astype(f32)
    ident = np.eye(128, dtype=f32)

    in_maps_b = [
        {"r": res_a[c]["r_out"], "ident": ident, "crep": crep}
        for c in range(NCORES)
    ]

    # ---- launch B
    res_b = run_b(in_maps_b)

    # ---- gather (undo the per-core row permutation)
    out = np.empty((B_TOT, 1), dtype=f32)
    for c in range(NCORES):
        o = res_b[c]["o_out"]  # (128, NBLK*4); row r = col*128 + p
        rows = o.T.reshape(RPC)
        out[srcs[c], 0] = rows
    return out
